# revision 10
# baseline (speedup 1.0000x reference)
"""Duration-based length regulation (KittenTTS LengthRegulator) on 8 trn2 NeuronCores.

For each batch b (one per core): phoneme t's feature row is repeated
clamp(durations[b,t],1) times along the frame axis; frames are zero-padded to
MAX_LEN = T*15 (pad rows are never written: the runners hand the kernel
pre-zeroed output buffers).

Raw-bass kernel (no TileContext): Tile's auto-dependency tracking daisy-chains
consecutive SWDGE scatters on the shared output tensor (each waits for the
previous one to fully drain), which serialized the baseline to ~6x the HBM
roofline. Here the five engine streams are synchronized manually. DMA
completion semaphores arrive ~2.5-3us after issue, so the load order is chosen
to put each consumer's gating tensor first on its ring:

  SP ring:    durations (gates everything), then the mask constants.
  ACT ring:   the matmul constant first (gates the PE), then features
              4x[128,512]; the ACT engine then builds the replicas for tiles
              2-3 (DVE covers tiles 0-1) so replication ends ~2x sooner and
              leaves DVE idle during descriptor generation (DVE SBUF traffic
              slows the SWDGE Q7 ring writes).
  PE:         exclusive global cumsum: strict-upper-tri matmul gives the
              within-column partial sums, all-ones matmul gives column sums.
  DVE:        per-pass additive terms hi_s = (dur & -(2s)) + OOB*((dur&s)==0),
              then the w=1,2 replica levels, then exc = partial + shifted
              column sums and offs_s = exc + hi_s (s in {1,2,4,8}, binary
              block decomposition; OOB pushes masked descriptors past
              bounds_check so the ucode skips them), then the w=4 level.
              Offsets are ordered right before the first scatter's gate;
              every DVE/ACT op bumps a counter sem and dependent ops wait on
              it (the pipelined engines do not order same-engine RAW).
  Pool:       a 4-byte SWDGE warm-up DMA (the first SWDGE op pays ~1.2us of
              ucode warm-up), then 16 indirect scatter DMAs ([128,1] offsets -
              the only offset shape the HW ucode supports) issued back-to-back
              in pass order s=4,8,2,1: the big passes queue most of the bytes
              early so the 16 SDMA engines never starve, while s=4 only needs
              the first two replica levels. One final wait for all 256
              completion increments.

Each output row is written exactly once -> DMA write traffic == sum(dur) rows
(~8 MB/core), which is the HBM-write roofline for this kernel.
"""

import sys

import numpy as np

if "/opt/trn_rl_repo" not in sys.path:
    sys.path.insert(0, "/opt/trn_rl_repo")

B, T, D = 8, 512, 512
MAX_DUR = 15
MAX_LEN = T * MAX_DUR  # 7680
P = 128
NT = T // P  # 4 duration columns / feature tiles
SBLK = [1, 2, 4, 8]  # pass sizes in offset-column order (issue order differs)
ISSUE = [4, 8, 2, 1]  # scatter issue order: feed the SDMA engines big passes first
OOB = 1 << 20  # pushed past bounds_check -> descriptor silently skipped
SMAX = 8
DVE_TILES = (0, 1)  # replica doubling on DVE
ACT_TILES = (2, 3)  # replica doubling on ACT

_CACHE = {}


def _host_constants():
    """Input-independent constant tensors shipped with every batch."""
    # tri[:, 0:128]: strict upper triangular ones (lhsT for exclusive cumsum
    # along partitions: (tri.T @ x)[p] = sum_{p'<p} x[p']).
    # tri[:, 128:256]: all ones (column sums, broadcast to every partition).
    tri = np.zeros((P, 2 * P), dtype=np.float32)
    tri[:, :P] = np.triu(np.ones((P, P), dtype=np.float32), k=1)
    tri[:, P:] = 1.0
    # ic[:, 0:16]:  -(2s) per pass column group (hi = dur & -(2s))
    # ic[:, 16:32]: s bit per pass column group (mask = dur & s)
    ic = np.zeros((P, 32), dtype=np.int32)
    for si, s in enumerate(SBLK):
        ic[:, si * NT : (si + 1) * NT] = -(2 * s)
        ic[:, 16 + si * NT : 16 + (si + 1) * NT] = s
    return tri, ic


def _build_nc():
    from concourse import bass, mybir
    from concourse.bacc import Bacc

    f32, i32 = mybir.dt.float32, mybir.dt.int32
    Alu = mybir.AluOpType

    nc = Bacc()
    feats = nc.declare_dram_parameter("features", [T, D], f32, isOutput=False)
    durs_mat = nc.declare_dram_parameter("durations_t", [P, NT], i32, isOutput=False)
    tri_c = nc.declare_dram_parameter("tri_const", [P, 2 * P], f32, isOutput=False)
    int_c = nc.declare_dram_parameter("int_const", [P, 32], i32, isOutput=False)
    out = nc.declare_dram_parameter("out", [MAX_LEN, D], f32, isOutput=True)

    # SBUF: replication tile j occupies cols [j*SMAX*D, (j+1)*SMAX*D);
    # replica r of row (j*128+p) sits at rep[p, j*SMAX*D + r*D : .. + D]
    rep = nc.alloc_sbuf_tensor("rep", [P, NT * SMAX * D], f32)
    dur_sb = nc.alloc_sbuf_tensor("dur_sb", [P, NT], i32)
    tri_sb = nc.alloc_sbuf_tensor("tri_sb", [P, 2 * P], f32)
    ic_sb = nc.alloc_sbuf_tensor("ic_sb", [P, 32], i32)
    durf = nc.alloc_sbuf_tensor("durf", [P, NT], f32)
    dur4 = nc.alloc_sbuf_tensor("dur4", [P, 4 * NT], i32)
    excf = nc.alloc_sbuf_tensor("excf", [P, NT], f32)
    offs = nc.alloc_sbuf_tensor("offs", [P, 4 * NT], i32)
    hi4 = nc.alloc_sbuf_tensor("hi4", [P, 4 * NT], i32)
    m4 = nc.alloc_sbuf_tensor("m4", [P, 4 * NT], i32)
    warm = nc.alloc_sbuf_tensor("warm", [1, 4], i32)
    ps = nc.alloc_psum_tensor("ps", [P, NT], f32)
    cs = nc.alloc_psum_tensor("cs", [P, NT], f32)

    s_dur = nc.alloc_semaphore("s_dur")  # durations load
    s_tri = nc.alloc_semaphore("s_tri")  # matmul-constant load
    s_ic = nc.alloc_semaphore("s_ic")  # mask-constant load
    s_f = [nc.alloc_semaphore(f"s_f{j}") for j in range(NT)]  # feature loads
    s_mm = nc.alloc_semaphore("s_mm")  # PE cumsum done
    s_v = nc.alloc_semaphore("s_v")  # DVE op counter
    s_a = nc.alloc_semaphore("s_a")  # ACT copy counter
    s_w = nc.alloc_semaphore("s_w")  # SWDGE warm-up DMA
    s_sc = nc.alloc_semaphore("s_sc")  # scatter DMA completions

    V_DURF = 2  # durf written (DVE op index on s_v)
    V_W1 = 10  # DVE w=1 replicas done
    V_W2 = 12  # DVE w=2 replicas done
    V_OFFS = 20  # all scatter offsets written
    V_W4 = 22  # DVE w=4 replicas done
    # (s_v, s_a) thresholds for the replica level pass s reads
    REP_DONE = {1: (0, 0), 2: (V_W1, 2), 4: (V_W2, 4), 8: (V_W4, 6)}

    with nc.Block(no_gpsimd_drain=True) as blk:

        @blk.sync
        def _(sync):
            sync.dma_start(out=dur_sb[:], in_=durs_mat[:, :]).then_inc(s_dur, 16)
            sync.dma_start(out=ic_sb[:], in_=int_c[:, :]).then_inc(s_ic, 16)

        @blk.scalar
        def _(scalar):
            scalar.dma_start(out=tri_sb[:], in_=tri_c[:, :]).then_inc(s_tri, 16)
            for j in range(NT):
                scalar.dma_start(
                    out=rep[:, j * SMAX * D : j * SMAX * D + D],
                    in_=feats[j * P : (j + 1) * P, :],
                ).then_inc(s_f[j], 16)

            # replicas for ACT_TILES: w1(t2), w1(t3), w2(t2), w2(t3), ...
            na = 0
            for w in (1, 2, 4):
                for j in ACT_TILES:
                    if w == 1:
                        scalar.wait_ge(s_f[j], 16)
                    else:
                        scalar.wait_ge(s_a, na - 1)  # same tile's previous level
                    base = j * SMAX * D
                    scalar.copy(
                        out=rep[:, base + w * D : base + 2 * w * D],
                        in_=rep[:, base : base + w * D],
                    ).then_inc(s_a, 1)
                    na += 1

        @blk.tensor
        def _(tensor):
            tensor.wait_ge(s_dur, 16)  # early wake from the idle stall
            tensor.wait_ge(s_v, V_DURF)
            tensor.wait_ge(s_tri, 16)
            tensor.matmul(ps[:, :], tri_sb[:, 0:P], durf[:, :], start=True, stop=True)
            tensor.matmul(
                cs[:, :], tri_sb[:, P : 2 * P], durf[:, :], start=True, stop=True
            ).then_inc(s_mm, 1)

        @blk.vector
        def _(vector):
            n = 0  # s_v value after each op below

            def op(inst):
                nonlocal n
                n += 1
                return inst.then_inc(s_v, 1)

            def dep(k):
                vector.wait_ge(s_v, k)

            # --- before the matmul lands: clamp, f32 view, per-pass hi terms
            vector.wait_ge(s_dur, 16)
            op(vector.tensor_scalar_max(out=dur4[:, 0:NT], in0=dur_sb[:], scalar1=1))  # 1
            dep(1)
            op(vector.tensor_copy(out=durf[:], in_=dur4[:, 0:NT]))  # 2 = V_DURF
            dep(1)
            op(vector.tensor_copy(out=dur4[:, NT : 2 * NT], in_=dur4[:, 0:NT]))  # 3
            dep(3)
            op(vector.tensor_copy(out=dur4[:, 2 * NT : 4 * NT], in_=dur4[:, 0 : 2 * NT]))  # 4
            vector.wait_ge(s_ic, 16)
            dep(4)
            op(vector.tensor_tensor(
                out=hi4[:], in0=dur4[:], in1=ic_sb[:, 0:16], op=Alu.bitwise_and
            ))  # 5
            dep(4)
            op(vector.tensor_tensor(
                out=m4[:], in0=dur4[:], in1=ic_sb[:, 16:32], op=Alu.bitwise_and
            ))  # 6
            dep(6)
            op(vector.tensor_scalar(
                out=m4[:], in0=m4[:], scalar1=0, scalar2=OOB,
                op0=Alu.is_equal, op1=Alu.mult,
            ))  # 7
            dep(7)
            op(vector.tensor_tensor(out=hi4[:], in0=hi4[:], in1=m4[:], op=Alu.add))  # 8

            # --- w=1,2 replicas for DVE_TILES (gate the first two passes)
            for j in DVE_TILES:
                vector.wait_ge(s_f[j], 16)
            for w in (1, 2):
                if w > 1:
                    dep(n)
                for j in DVE_TILES:
                    base = j * SMAX * D
                    op(vector.tensor_copy(
                        out=rep[:, base + w * D : base + 2 * w * D],
                        in_=rep[:, base : base + w * D],
                    ))  # 9-10 (w1), 11-12 (w2)

            # --- after the matmul: exc, then offs = exc4 + hi4
            vector.wait_ge(s_mm, 1)
            op(vector.tensor_copy(out=excf[:], in_=ps[:, :]))  # 13
            for sh in range(1, NT):
                dep(n)
                op(vector.tensor_tensor(
                    out=excf[:, sh:NT], in0=excf[:, sh:NT],
                    in1=cs[:, 0 : NT - sh], op=Alu.add,
                ))  # 14,15,16
            dep(16)
            op(vector.tensor_copy(out=offs[:, 0:NT], in_=excf[:]))  # 17 (f32->i32)
            dep(17)
            op(vector.tensor_copy(out=offs[:, NT : 2 * NT], in_=offs[:, 0:NT]))  # 18
            dep(18)
            op(vector.tensor_copy(out=offs[:, 2 * NT : 4 * NT], in_=offs[:, 0 : 2 * NT]))  # 19
            dep(19)
            op(vector.tensor_tensor(
                out=offs[:], in0=offs[:], in1=hi4[:], op=Alu.add
            ))  # 20 = V_OFFS

            # --- w=4 replicas for DVE_TILES (only the s=8 pass needs them)
            dep(V_W2)
            for j in DVE_TILES:
                base = j * SMAX * D
                op(vector.tensor_copy(
                    out=rep[:, base + 4 * D : base + 8 * D],
                    in_=rep[:, base : base + 4 * D],
                ))  # 21-22

        @blk.gpsimd
        def _(gpsimd):
            # SWDGE warm-up: the first SWDGE op pays ~1.2us of Q7 setup
            gpsimd.dma_start(out=warm[:], in_=int_c[0:1, 0:4]).then_inc(s_w, 16)
            bregs = {s_: gpsimd.to_reg(MAX_LEN - s_) for s_ in SBLK}
            for j in range(NT):
                gpsimd.wait_ge(s_f[j], 16)
            gpsimd.wait_ge(s_v, V_OFFS)
            for s_ in ISSUE:
                si = SBLK.index(s_)
                v_need, a_need = REP_DONE[s_]
                if v_need and v_need > V_OFFS:
                    gpsimd.wait_ge(s_v, v_need)
                if a_need:
                    gpsimd.wait_ge(s_a, a_need)
                for j in range(NT):
                    gpsimd.indirect_dma_start(
                        out=out[:, :],
                        out_offset=bass.IndirectOffsetOnAxis(
                            ap=offs[:, si * NT + j : si * NT + j + 1], axis=0
                        ),
                        in_=rep[:, j * SMAX * D : j * SMAX * D + s_ * D],
                        in_offset=None,
                        bounds_check=bregs[s_],
                        oob_is_err=False,
                    ).then_inc(s_sc, 16)
            gpsimd.wait_ge(s_sc, 16 * 4 * NT)  # all 16 scatters drained
            gpsimd.wait_ge(s_w, 16)

    nc.compile()
    return nc


def _get_nc():
    if "nc" not in _CACHE:
        _CACHE["nc"] = _build_nc()
    return _CACHE["nc"]


def _run(features, durations, trace=False):
    """features (B,T,D) f32, durations (B,T) i32 -> (out (B,MAX_LEN,D) f32, results)."""
    from concourse.bass_utils import run_bass_kernel_spmd

    nc = _get_nc()
    tri, ic = _host_constants()
    in_maps = []
    for b in range(B):
        dmat = np.ascontiguousarray(durations[b].reshape(NT, P).T)  # [P, NT]
        in_maps.append(
            {
                "features": np.ascontiguousarray(features[b]),
                "durations_t": dmat,
                "tri_const": tri,
                "int_const": ic,
            }
        )
    kwargs = {}
    if trace:
        kwargs = dict(trace=True, trace_cores=list(range(B)), stitch_traces=False)
    res = run_bass_kernel_spmd(nc, in_maps, core_ids=list(range(B)), **kwargs)
    outs = np.stack([res.results[b]["out"] for b in range(B)])
    return outs.astype(np.float32, copy=False), res


def kernel(features, durations):
    features = np.asarray(features, dtype=np.float32)
    durations = np.asarray(durations, dtype=np.int32)
    outs, _ = _run(features, durations, trace=False)
    return outs


if __name__ == "__main__":
    feats = np.random.randn(B, T, D).astype(np.float32)
    durs = np.random.randint(0, 16, size=(B, T)).astype(np.int32)
    out = kernel(feats, durs)
    print("out", out.shape, out.dtype)


# revision 11
# speedup vs baseline: 1.0376x; 1.0376x over previous
"""Duration-based length regulation (KittenTTS LengthRegulator) on 8 trn2 NeuronCores.

For each batch b (one per core): phoneme t's feature row is repeated
clamp(durations[b,t],1) times along the frame axis; frames are zero-padded to
MAX_LEN = T*15 (pad rows are never written: the runners hand the kernel
pre-zeroed output buffers).

Raw-bass kernel (no TileContext): Tile's auto-dependency tracking daisy-chains
consecutive SWDGE scatters on the shared output tensor (each waits for the
previous one to fully drain), which serialized the baseline to ~6x the HBM
roofline. Five engine streams with manual semaphores instead. DMA completion
semaphores arrive ~2-3us after the transfer, and every cross-engine handoff
costs 1-2us, so the schedule minimizes hops on the path to the first scatter:

  SP ring:    durations (int + f32 views), mask constants, feature tiles 0-1.
  ACT ring:   the matmul constant first (gates the PE), feature tiles 2-3;
              the ACT engine then builds replicas for tiles 2-3.
  PE:         exclusive global cumsum straight from the loaded f32 durations:
              strict-upper-tri matmul (within-column partial sums) + all-ones
              matmul (column sums). No DVE dependency.
  DVE:        per-pass additive terms hi_s = (dur & -(2s)) + OOB*((dur&s)==0)
              for s in {1,2,4,8} (binary block decomposition; OOB pushes masked
              descriptors past bounds_check so the ucode skips them), replica
              levels for tiles 0-1 interleaved with the offset chain
              offs_s = (partial + shifted column sums) + hi_s.
              Every DVE/ACT op bumps a counter sem and dependent ops wait on it
              (the pipelined engines do not order same-engine RAW).
  Pool:       a 4-byte SWDGE warm-up DMA (the first SWDGE op pays ~1us of
              ucode warm-up), then 16 indirect scatter DMAs ([128,1] offsets -
              the only offset shape the HW ucode supports) issued back-to-back
              in pass order s=4,8,2,1 so the 16 SDMA engines never starve.
              One final wait for all 256 completion increments.

Each output row is written exactly once -> DMA write traffic == sum(dur) rows
(~8 MB/core), which is the HBM-write roofline for this kernel.
"""

import sys

import numpy as np

if "/opt/trn_rl_repo" not in sys.path:
    sys.path.insert(0, "/opt/trn_rl_repo")

B, T, D = 8, 512, 512
MAX_DUR = 15
MAX_LEN = T * MAX_DUR  # 7680
P = 128
NT = T // P  # 4 duration columns / feature tiles
SBLK = [1, 2, 4, 8]  # pass sizes in offset-column order (issue order differs)
ISSUE = [4, 8, 2, 1]  # scatter issue order: feed the SDMA engines big passes first
OOB = 1 << 20  # pushed past bounds_check -> descriptor silently skipped
SMAX = 8
DVE_TILES = (0, 1)  # replica doubling on DVE
ACT_TILES = (2, 3)  # replica doubling on ACT

_CACHE = {}


def _host_constants():
    """Input-independent constant tensors shipped with every batch."""
    # tri[:, 0:128]: strict upper triangular ones (lhsT for exclusive cumsum
    # along partitions: (tri.T @ x)[p] = sum_{p'<p} x[p']).
    # tri[:, 128:256]: all ones (column sums, broadcast to every partition).
    tri = np.zeros((P, 2 * P), dtype=np.float32)
    tri[:, :P] = np.triu(np.ones((P, P), dtype=np.float32), k=1)
    tri[:, P:] = 1.0
    # ic[:, 0:16]:  -(2s) per pass column group (hi = dur & -(2s))
    # ic[:, 16:32]: s bit per pass column group (mask = dur & s)
    ic = np.zeros((P, 32), dtype=np.int32)
    for si, s in enumerate(SBLK):
        ic[:, si * NT : (si + 1) * NT] = -(2 * s)
        ic[:, 16 + si * NT : 16 + (si + 1) * NT] = s
    return tri, ic


def _build_nc():
    from concourse import bass, mybir
    from concourse.bacc import Bacc

    f32, i32 = mybir.dt.float32, mybir.dt.int32
    Alu = mybir.AluOpType

    nc = Bacc()
    feats = nc.declare_dram_parameter("features", [T, D], f32, isOutput=False)
    durs_mat = nc.declare_dram_parameter("durations_t", [P, NT], i32, isOutput=False)
    durs_f = nc.declare_dram_parameter("durations_f", [P, NT], f32, isOutput=False)
    tri_c = nc.declare_dram_parameter("tri_const", [P, 2 * P], f32, isOutput=False)
    int_c = nc.declare_dram_parameter("int_const", [P, 32], i32, isOutput=False)
    out = nc.declare_dram_parameter("out", [MAX_LEN, D], f32, isOutput=True)

    # SBUF: replication tile j occupies cols [j*SMAX*D, (j+1)*SMAX*D);
    # replica r of row (j*128+p) sits at rep[p, j*SMAX*D + r*D : .. + D]
    rep = nc.alloc_sbuf_tensor("rep", [P, NT * SMAX * D], f32)
    dur_sb = nc.alloc_sbuf_tensor("dur_sb", [P, NT], i32)
    durf = nc.alloc_sbuf_tensor("durf", [P, NT], f32)
    tri_sb = nc.alloc_sbuf_tensor("tri_sb", [P, 2 * P], f32)
    ic_sb = nc.alloc_sbuf_tensor("ic_sb", [P, 32], i32)
    dur4 = nc.alloc_sbuf_tensor("dur4", [P, 4 * NT], i32)
    excf = nc.alloc_sbuf_tensor("excf", [P, NT], f32)
    offs = nc.alloc_sbuf_tensor("offs", [P, 4 * NT], i32)
    hi4 = nc.alloc_sbuf_tensor("hi4", [P, 4 * NT], i32)
    m4 = nc.alloc_sbuf_tensor("m4", [P, 4 * NT], i32)
    warm = nc.alloc_sbuf_tensor("warm", [1, 4], i32)
    ps = nc.alloc_psum_tensor("ps", [P, NT], f32)
    cs = nc.alloc_psum_tensor("cs", [P, NT], f32)

    s_dur = nc.alloc_semaphore("s_dur")  # int durations load
    s_df = nc.alloc_semaphore("s_df")  # f32 durations load
    s_tri = nc.alloc_semaphore("s_tri")  # matmul-constant load
    s_ic = nc.alloc_semaphore("s_ic")  # mask-constant load
    s_f = [nc.alloc_semaphore(f"s_f{j}") for j in range(NT)]  # feature loads
    s_mm = nc.alloc_semaphore("s_mm")  # PE cumsum done
    s_v = nc.alloc_semaphore("s_v")  # DVE op counter
    s_a = nc.alloc_semaphore("s_a")  # ACT copy counter
    s_w = nc.alloc_semaphore("s_w")  # SWDGE warm-up DMA
    s_sc = nc.alloc_semaphore("s_sc")  # scatter DMA completions

    # s_v milestones (see the vector stream): 6 pre-ops, then
    # w1(t0)=7, w1(t1)=8, w2(t0)=9, offsets chain 10..17, w2(t1)=18,
    # w4(t0)=19, w4(t1)=20
    V_OFFS = 17
    V_W2 = 18
    V_W4 = 20

    with nc.Block(no_gpsimd_drain=True) as blk:

        @blk.sync
        def _(sync):
            sync.dma_start(out=dur_sb[:], in_=durs_mat[:, :]).then_inc(s_dur, 16)
            sync.dma_start(out=durf[:], in_=durs_f[:, :]).then_inc(s_df, 16)
            sync.dma_start(out=ic_sb[:], in_=int_c[:, :]).then_inc(s_ic, 16)
            for j in DVE_TILES:
                sync.dma_start(
                    out=rep[:, j * SMAX * D : j * SMAX * D + D],
                    in_=feats[j * P : (j + 1) * P, :],
                ).then_inc(s_f[j], 16)

        @blk.scalar
        def _(scalar):
            scalar.dma_start(out=tri_sb[:], in_=tri_c[:, :]).then_inc(s_tri, 16)
            for j in ACT_TILES:
                scalar.dma_start(
                    out=rep[:, j * SMAX * D : j * SMAX * D + D],
                    in_=feats[j * P : (j + 1) * P, :],
                ).then_inc(s_f[j], 16)

            # replicas for ACT_TILES: w1(t2), w1(t3), w2(t2), w2(t3), ...
            na = 0
            for w in (1, 2, 4):
                for j in ACT_TILES:
                    if w == 1:
                        scalar.wait_ge(s_f[j], 16)
                    else:
                        scalar.wait_ge(s_a, na - 1)  # same tile's previous level
                    base = j * SMAX * D
                    scalar.copy(
                        out=rep[:, base + w * D : base + 2 * w * D],
                        in_=rep[:, base : base + w * D],
                    ).then_inc(s_a, 1)
                    na += 1

        @blk.tensor
        def _(tensor):
            tensor.wait_ge(s_df, 16)
            tensor.wait_ge(s_tri, 16)
            tensor.matmul(ps[:, :], tri_sb[:, 0:P], durf[:, :], start=True, stop=True)
            tensor.matmul(
                cs[:, :], tri_sb[:, P : 2 * P], durf[:, :], start=True, stop=True
            ).then_inc(s_mm, 1)

        @blk.vector
        def _(vector):
            n = 0  # s_v value after each op below

            def op(inst):
                nonlocal n
                n += 1
                return inst.then_inc(s_v, 1)

            def dep(k):
                vector.wait_ge(s_v, k)

            def rep_copy(j, w):
                base = j * SMAX * D
                return vector.tensor_copy(
                    out=rep[:, base + w * D : base + 2 * w * D],
                    in_=rep[:, base : base + w * D],
                )

            # --- per-pass hi terms (independent of the cumsum)
            vector.wait_ge(s_dur, 16)
            op(vector.tensor_copy(out=dur4[:, 0:NT], in_=dur_sb[:]))  # 1
            dep(1)
            op(vector.tensor_copy(out=dur4[:, NT : 2 * NT], in_=dur4[:, 0:NT]))  # 2
            dep(2)
            op(vector.tensor_copy(out=dur4[:, 2 * NT : 4 * NT], in_=dur4[:, 0 : 2 * NT]))  # 3
            vector.wait_ge(s_ic, 16)
            dep(3)
            op(vector.tensor_tensor(
                out=hi4[:], in0=dur4[:], in1=ic_sb[:, 0:16], op=Alu.bitwise_and
            ))  # 4
            dep(3)
            op(vector.tensor_tensor(
                out=m4[:], in0=dur4[:], in1=ic_sb[:, 16:32], op=Alu.bitwise_and
            ))  # 5
            dep(5)
            op(vector.tensor_scalar(
                out=m4[:], in0=m4[:], scalar1=0, scalar2=OOB,
                op0=Alu.is_equal, op1=Alu.mult,
            ))  # 6
            dep(6)
            op(vector.tensor_tensor(out=hi4[:], in0=hi4[:], in1=m4[:], op=Alu.add))  # 6b -> n=7? (see below)

            # NOTE: op count so far is 7; replica/offset milestones follow
            vector.wait_ge(s_f[0], 16)
            op(rep_copy(0, 1))  # 8
            vector.wait_ge(s_f[1], 16)
            op(rep_copy(1, 1))  # 9
            dep(8)
            op(rep_copy(0, 2))  # 10

            # --- offsets: exc then offs = exc4 + hi4
            vector.wait_ge(s_mm, 1)
            op(vector.tensor_copy(out=excf[:], in_=ps[:, :]))  # 11
            for sh in range(1, NT):
                dep(n)
                op(vector.tensor_tensor(
                    out=excf[:, sh:NT], in0=excf[:, sh:NT],
                    in1=cs[:, 0 : NT - sh], op=Alu.add,
                ))  # 12,13,14
            dep(14)
            op(vector.tensor_copy(out=offs[:, 0:NT], in_=excf[:]))  # 15 (f32->i32)
            dep(15)
            op(vector.tensor_copy(out=offs[:, NT : 2 * NT], in_=offs[:, 0:NT]))  # 16
            dep(16)
            op(vector.tensor_copy(out=offs[:, 2 * NT : 4 * NT], in_=offs[:, 0 : 2 * NT]))  # 17
            dep(17)
            op(vector.tensor_tensor(
                out=offs[:], in0=offs[:], in1=hi4[:], op=Alu.add
            ))  # 18 == V_OFFS

            dep(9)
            op(rep_copy(1, 2))  # 19 == V_W2
            dep(10)
            op(rep_copy(0, 4))  # 20
            dep(19)
            op(rep_copy(1, 4))  # 21 == V_W4

        @blk.gpsimd
        def _(gpsimd):
            # SWDGE warm-up: the first SWDGE op pays ~1us of Q7 setup
            gpsimd.dma_start(out=warm[:], in_=int_c[0:1, 0:4]).then_inc(s_w, 16)
            bregs = {s_: gpsimd.to_reg(MAX_LEN - s_) for s_ in SBLK}
            for j in range(NT):
                gpsimd.wait_ge(s_f[j], 16)
            gpsimd.wait_ge(s_v, V_OFFS + 1)
            gates = {4: (V_W2 + 1, 4), 8: (V_W4 + 1, 6), 2: (0, 2), 1: (0, 0)}
            for s_ in ISSUE:
                si = SBLK.index(s_)
                v_need, a_need = gates[s_]
                if v_need:
                    gpsimd.wait_ge(s_v, v_need)
                if a_need:
                    gpsimd.wait_ge(s_a, a_need)
                for j in range(NT):
                    gpsimd.indirect_dma_start(
                        out=out[:, :],
                        out_offset=bass.IndirectOffsetOnAxis(
                            ap=offs[:, si * NT + j : si * NT + j + 1], axis=0
                        ),
                        in_=rep[:, j * SMAX * D : j * SMAX * D + s_ * D],
                        in_offset=None,
                        bounds_check=bregs[s_],
                        oob_is_err=False,
                    ).then_inc(s_sc, 16)
            gpsimd.wait_ge(s_sc, 16 * 4 * NT)  # all 16 scatters drained
            gpsimd.wait_ge(s_w, 16)

    nc.compile()
    return nc


def _get_nc():
    if "nc" not in _CACHE:
        _CACHE["nc"] = _build_nc()
    return _CACHE["nc"]


def _run(features, durations, trace=False):
    """features (B,T,D) f32, durations (B,T) i32 -> (out (B,MAX_LEN,D) f32, results)."""
    from concourse.bass_utils import run_bass_kernel_spmd

    nc = _get_nc()
    tri, ic = _host_constants()
    in_maps = []
    for b in range(B):
        durc = np.maximum(durations[b], 1)  # clamp(min=1), as in forward()
        dmat = np.ascontiguousarray(durc.reshape(NT, P).T)  # [P, NT]
        in_maps.append(
            {
                "features": np.ascontiguousarray(features[b]),
                "durations_t": dmat,
                "durations_f": dmat.astype(np.float32),
                "tri_const": tri,
                "int_const": ic,
            }
        )
    kwargs = {}
    if trace:
        kwargs = dict(trace=True, trace_cores=list(range(B)), stitch_traces=False)
    res = run_bass_kernel_spmd(nc, in_maps, core_ids=list(range(B)), **kwargs)
    outs = np.stack([res.results[b]["out"] for b in range(B)])
    return outs.astype(np.float32, copy=False), res


def kernel(features, durations):
    features = np.asarray(features, dtype=np.float32)
    durations = np.asarray(durations, dtype=np.int32)
    outs, _ = _run(features, durations, trace=False)
    return outs


if __name__ == "__main__":
    feats = np.random.randn(B, T, D).astype(np.float32)
    durs = np.random.randint(0, 16, size=(B, T)).astype(np.int32)
    out = kernel(feats, durs)
    print("out", out.shape, out.dtype)


# revision 14
# speedup vs baseline: 1.0814x; 1.0422x over previous
"""Duration-based length regulation (KittenTTS LengthRegulator) on 8 trn2 NeuronCores.

For each batch b (one per core): phoneme t's feature row is repeated
clamp(durations[b,t],1) times along the frame axis; frames are zero-padded to
MAX_LEN = T*15 (pad rows are never written: the runners hand the kernel
pre-zeroed output buffers).

Raw-bass kernel (no TileContext): Tile's auto-dependency tracking daisy-chains
consecutive SWDGE scatters on the shared output tensor (each waits for the
previous one to fully drain), which serialized the baseline to ~6x the HBM
roofline. Five engine streams with manual semaphores instead. DMA completion
semaphores arrive ~2-3us after the transfer, and every cross-engine handoff
costs 1-2us, so the schedule minimizes hops on the path to the first scatter:

  SP ring:    durations (int + f32 views), mask constants, feature tiles 0-1.
  ACT ring:   the matmul constant first (gates the PE), feature tiles 2-3;
              the ACT engine then builds replicas for tiles 2-3.
  PE:         exclusive global cumsum straight from the loaded f32 durations:
              strict-upper-tri matmul (within-column partial sums) + all-ones
              matmul (column sums). No DVE dependency.
  DVE:        per-pass additive terms hi_s = (dur & -(2s)) + OOB*((dur&s)==0)
              for s in {1,2,4,8} (binary block decomposition; OOB pushes masked
              descriptors past bounds_check so the ucode skips them), replica
              levels for tiles 0-1 interleaved with the offset chain
              offs_s = (partial + shifted column sums) + hi_s.
              Every DVE/ACT op bumps a counter sem and dependent ops wait on it
              (the pipelined engines do not order same-engine RAW).
  Pool:       a 4-byte SWDGE warm-up DMA (the first SWDGE op pays ~1us of
              ucode warm-up), then 16 indirect scatter DMAs ([128,1] offsets -
              the only offset shape the HW ucode supports) issued back-to-back
              in pass order s=4,8,2,1 so the 16 SDMA engines never starve.
              One final wait for all 256 completion increments.

Each output row is written exactly once -> DMA write traffic == sum(dur) rows
(~8 MB/core), which is the HBM-write roofline for this kernel.
"""

import sys

import numpy as np

if "/opt/trn_rl_repo" not in sys.path:
    sys.path.insert(0, "/opt/trn_rl_repo")

B, T, D = 8, 512, 512
MAX_DUR = 15
MAX_LEN = T * MAX_DUR  # 7680
P = 128
NT = T // P  # 4 duration columns / feature tiles
SBLK = [1, 2, 4, 8]  # pass sizes in offset-column order (issue order differs)
ISSUE = [4, 8, 2, 1]  # scatter issue order: feed the SDMA engines big passes first
OOB = 1 << 20  # pushed past bounds_check -> descriptor silently skipped
SMAX = 8
DVE_TILES = (0, 1)  # replica doubling on DVE
ACT_TILES = (2, 3)  # replica doubling on ACT

_CACHE = {}


def _host_constants():
    """Input-independent constant tensors shipped with every batch."""
    # tri[:, 0:128]: strict upper triangular ones (lhsT for exclusive cumsum
    # along partitions: (tri.T @ x)[p] = sum_{p'<p} x[p']).
    # tri[:, 128:256]: all ones (column sums, broadcast to every partition).
    tri = np.zeros((P, 2 * P), dtype=np.float32)
    tri[:, :P] = np.triu(np.ones((P, P), dtype=np.float32), k=1)
    tri[:, P:] = 1.0
    # ic[:, 0:16]:  -(2s) per pass column group (hi = dur & -(2s))
    # ic[:, 16:32]: s bit per pass column group (mask = dur & s)
    ic = np.zeros((P, 32), dtype=np.int32)
    for si, s in enumerate(SBLK):
        ic[:, si * NT : (si + 1) * NT] = -(2 * s)
        ic[:, 16 + si * NT : 16 + (si + 1) * NT] = s
    return tri, ic


def _build_nc():
    from concourse import bass, mybir
    from concourse.bacc import Bacc

    f32, i32 = mybir.dt.float32, mybir.dt.int32
    Alu = mybir.AluOpType

    nc = Bacc()
    feats = nc.declare_dram_parameter("features", [T, D], f32, isOutput=False)
    durs_mat = nc.declare_dram_parameter("durations_t", [P, NT], i32, isOutput=False)
    durs_f = nc.declare_dram_parameter("durations_f", [P, NT], f32, isOutput=False)
    tri_c = nc.declare_dram_parameter("tri_const", [P, 2 * P], f32, isOutput=False)
    int_c = nc.declare_dram_parameter("int_const", [P, 32], i32, isOutput=False)
    out = nc.declare_dram_parameter("out", [MAX_LEN, D], f32, isOutput=True)

    # SBUF: replication tile j occupies cols [j*SMAX*D, (j+1)*SMAX*D);
    # replica r of row (j*128+p) sits at rep[p, j*SMAX*D + r*D : .. + D]
    rep = nc.alloc_sbuf_tensor("rep", [P, NT * SMAX * D], f32)
    dur_sb = nc.alloc_sbuf_tensor("dur_sb", [P, NT], i32)
    durf = nc.alloc_sbuf_tensor("durf", [P, NT], f32)
    tri_sb = nc.alloc_sbuf_tensor("tri_sb", [P, 2 * P], f32)
    ic_sb = nc.alloc_sbuf_tensor("ic_sb", [P, 32], i32)
    dur4 = nc.alloc_sbuf_tensor("dur4", [P, 4 * NT], i32)
    excf = nc.alloc_sbuf_tensor("excf", [P, NT], f32)
    offs = nc.alloc_sbuf_tensor("offs", [P, 4 * NT], i32)
    hi4 = nc.alloc_sbuf_tensor("hi4", [P, 4 * NT], i32)
    m4 = nc.alloc_sbuf_tensor("m4", [P, 4 * NT], i32)
    warm = nc.alloc_sbuf_tensor("warm", [2, 1], i32)
    warmd = nc.alloc_sbuf_tensor("warmd", [2, D], f32)
    ps = nc.alloc_psum_tensor("ps", [P, NT], f32)
    cs = nc.alloc_psum_tensor("cs", [P, NT], f32)

    s_dur = nc.alloc_semaphore("s_dur")  # int durations load
    s_df = nc.alloc_semaphore("s_df")  # f32 durations load
    s_tri = nc.alloc_semaphore("s_tri")  # matmul-constant load
    s_ic = nc.alloc_semaphore("s_ic")  # mask-constant load
    s_f = [nc.alloc_semaphore(f"s_f{j}") for j in range(NT)]  # feature loads
    s_mm = nc.alloc_semaphore("s_mm")  # PE cumsum done
    s_v = nc.alloc_semaphore("s_v")  # DVE op counter
    s_a = nc.alloc_semaphore("s_a")  # ACT copy counter
    s_w = nc.alloc_semaphore("s_w")  # SWDGE warm-up DMA
    s_p = nc.alloc_semaphore("s_p")  # Pool memset counter
    s_sc = nc.alloc_semaphore("s_sc")  # scatter DMA completions

    # s_v milestones (see the vector stream)
    V_OFFS = 15
    V_W1 = 17
    V_W2 = 19
    V_W4 = 21

    with nc.Block(no_gpsimd_drain=True) as blk:

        @blk.sync
        def _(sync):
            sync.dma_start(out=dur_sb[:], in_=durs_mat[:, :]).then_inc(s_dur, 16)
            sync.dma_start(out=durf[:], in_=durs_f[:, :]).then_inc(s_df, 16)
            sync.dma_start(out=ic_sb[:], in_=int_c[:, :]).then_inc(s_ic, 16)
            for j in DVE_TILES:
                sync.dma_start(
                    out=rep[:, j * SMAX * D : j * SMAX * D + D],
                    in_=feats[j * P : (j + 1) * P, :],
                ).then_inc(s_f[j], 16)

        @blk.scalar
        def _(scalar):
            scalar.dma_start(out=tri_sb[:], in_=tri_c[:, :]).then_inc(s_tri, 16)
            for j in ACT_TILES:
                scalar.dma_start(
                    out=rep[:, j * SMAX * D : j * SMAX * D + D],
                    in_=feats[j * P : (j + 1) * P, :],
                ).then_inc(s_f[j], 16)

            # replicas for ACT_TILES: w1(t2), w1(t3), w2(t2), w2(t3), ...
            na = 0
            for w in (1, 2, 4):
                for j in ACT_TILES:
                    if w == 1:
                        scalar.wait_ge(s_f[j], 16)
                    else:
                        scalar.wait_ge(s_a, na - 1)  # same tile's previous level
                    base = j * SMAX * D
                    scalar.copy(
                        out=rep[:, base + w * D : base + 2 * w * D],
                        in_=rep[:, base : base + w * D],
                    ).then_inc(s_a, 1)
                    na += 1

        @blk.tensor
        def _(tensor):
            tensor.wait_ge(s_df, 16)
            tensor.wait_ge(s_tri, 16)
            tensor.matmul(ps[:, :], tri_sb[:, 0:P], durf[:, :], start=True, stop=True)
            tensor.matmul(
                cs[:, :], tri_sb[:, P : 2 * P], durf[:, :], start=True, stop=True
            ).then_inc(s_mm, 1)

        @blk.vector
        def _(vector):
            n = 0  # s_v value after each op below

            def op(inst):
                nonlocal n
                n += 1
                return inst.then_inc(s_v, 1)

            def dep(k):
                vector.wait_ge(s_v, k)

            def rep_copy(j, w):
                base = j * SMAX * D
                return vector.tensor_copy(
                    out=rep[:, base + w * D : base + 2 * w * D],
                    in_=rep[:, base : base + w * D],
                )

            # --- replicated int durations (feed the per-pass hi terms)
            vector.wait_ge(s_dur, 16)
            op(vector.tensor_copy(out=dur4[:, 0:NT], in_=dur_sb[:]))  # 1
            dep(1)
            op(vector.tensor_copy(out=dur4[:, NT : 2 * NT], in_=dur4[:, 0:NT]))  # 2
            dep(2)
            op(vector.tensor_copy(out=dur4[:, 2 * NT : 4 * NT], in_=dur4[:, 0 : 2 * NT]))  # 3

            # --- offsets chain straight off the matmul
            vector.wait_ge(s_mm, 1)
            op(vector.tensor_copy(out=excf[:], in_=ps[:, :]))  # 4
            for sh in range(1, NT):
                dep(n)
                op(vector.tensor_tensor(
                    out=excf[:, sh:NT], in0=excf[:, sh:NT],
                    in1=cs[:, 0 : NT - sh], op=Alu.add,
                ))  # 5,6,7
            dep(7)
            op(vector.tensor_copy(out=offs[:, 0:NT], in_=excf[:]))  # 8 (f32->i32)
            dep(8)
            op(vector.tensor_copy(out=offs[:, NT : 2 * NT], in_=offs[:, 0:NT]))  # 9
            dep(9)
            op(vector.tensor_copy(out=offs[:, 2 * NT : 4 * NT], in_=offs[:, 0 : 2 * NT]))  # 10
            vector.wait_ge(s_ic, 16)
            dep(3)
            op(vector.tensor_tensor(
                out=hi4[:], in0=dur4[:], in1=ic_sb[:, 0:16], op=Alu.bitwise_and
            ))  # 11
            dep(3)
            op(vector.tensor_tensor(
                out=m4[:], in0=dur4[:], in1=ic_sb[:, 16:32], op=Alu.bitwise_and
            ))  # 12
            dep(12)
            op(vector.tensor_scalar(
                out=m4[:], in0=m4[:], scalar1=0, scalar2=OOB,
                op0=Alu.is_equal, op1=Alu.mult,
            ))  # 13
            dep(13)
            op(vector.tensor_tensor(out=hi4[:], in0=hi4[:], in1=m4[:], op=Alu.add))  # 14
            dep(14)
            op(vector.tensor_tensor(
                out=offs[:], in0=offs[:], in1=hi4[:], op=Alu.add
            ))  # 15 == V_OFFS

            # --- replicas for DVE_TILES
            vector.wait_ge(s_f[0], 16)
            op(rep_copy(0, 1))  # 16
            vector.wait_ge(s_f[1], 16)
            op(rep_copy(1, 1))  # 17
            dep(16)
            op(rep_copy(0, 2))  # 18
            dep(17)
            op(rep_copy(1, 2))  # 19 == V_W2
            dep(18)
            op(rep_copy(0, 4))  # 20
            dep(19)
            op(rep_copy(1, 4))  # 21 == V_W4

        @blk.gpsimd
        def _(gpsimd):
            # SWDGE indirect-ucode warm-up: 2 descriptors, both pushed past
            # bounds_check=0 -> nothing is written, but the Q7 indirect kernel
            # is hot before the real scatters
            gpsimd.memset(warm[:], OOB).then_inc(s_p, 1)
            gpsimd.memset(warmd[:], 0.0).then_inc(s_p, 1)
            gpsimd.wait_ge(s_p, 2)
            gpsimd.indirect_dma_start(
                out=out[:, :],
                out_offset=bass.IndirectOffsetOnAxis(ap=warm[0:2, 0:1], axis=0),
                in_=warmd[0:2, 0:D],
                in_offset=None,
                bounds_check=nc.gpsimd.to_reg(0),
                oob_is_err=False,
            ).then_inc(s_w, 16)
            bregs = {s_: gpsimd.to_reg(MAX_LEN - s_) for s_ in SBLK}
            for j in range(NT):
                gpsimd.wait_ge(s_f[j], 16)
            gpsimd.wait_ge(s_v, V_OFFS)
            gates = {2: (V_W1, 2), 4: (V_W2, 4), 8: (V_W4, 6), 1: (0, 0)}
            for s_ in ISSUE:
                si = SBLK.index(s_)
                v_need, a_need = gates[s_]
                if v_need:
                    gpsimd.wait_ge(s_v, v_need)
                if a_need:
                    gpsimd.wait_ge(s_a, a_need)
                for j in range(NT):
                    gpsimd.indirect_dma_start(
                        out=out[:, :],
                        out_offset=bass.IndirectOffsetOnAxis(
                            ap=offs[:, si * NT + j : si * NT + j + 1], axis=0
                        ),
                        in_=rep[:, j * SMAX * D : j * SMAX * D + s_ * D],
                        in_offset=None,
                        bounds_check=bregs[s_],
                        oob_is_err=False,
                    ).then_inc(s_sc, 16)
            gpsimd.wait_ge(s_sc, 16 * 4 * NT)  # all 16 scatters drained
            gpsimd.wait_ge(s_w, 16)

    nc.compile()
    return nc


def _get_nc():
    if "nc" not in _CACHE:
        _CACHE["nc"] = _build_nc()
    return _CACHE["nc"]


def _run(features, durations, trace=False):
    """features (B,T,D) f32, durations (B,T) i32 -> (out (B,MAX_LEN,D) f32, results)."""
    from concourse.bass_utils import run_bass_kernel_spmd

    nc = _get_nc()
    tri, ic = _host_constants()
    in_maps = []
    for b in range(B):
        durc = np.maximum(durations[b], 1)  # clamp(min=1), as in forward()
        dmat = np.ascontiguousarray(durc.reshape(NT, P).T)  # [P, NT]
        in_maps.append(
            {
                "features": np.ascontiguousarray(features[b]),
                "durations_t": dmat,
                "durations_f": dmat.astype(np.float32),
                "tri_const": tri,
                "int_const": ic,
            }
        )
    kwargs = {}
    if trace:
        kwargs = dict(trace=True, trace_cores=list(range(B)), stitch_traces=False)
    res = run_bass_kernel_spmd(nc, in_maps, core_ids=list(range(B)), **kwargs)
    outs = np.stack([res.results[b]["out"] for b in range(B)])
    return outs.astype(np.float32, copy=False), res


def kernel(features, durations):
    features = np.asarray(features, dtype=np.float32)
    durations = np.asarray(durations, dtype=np.int32)
    outs, _ = _run(features, durations, trace=False)
    return outs


if __name__ == "__main__":
    feats = np.random.randn(B, T, D).astype(np.float32)
    durs = np.random.randint(0, 16, size=(B, T)).astype(np.int32)
    out = kernel(feats, durs)
    print("out", out.shape, out.dtype)
